# revision 1
# baseline (speedup 1.0000x reference)
"""Deformable Conv2d (adaptive, modulated) for Trainium2 — 8-core SPMD Bass kernel.

Strategy
--------
Shard (batch, H) into 8 shards: core = b*4 + hchunk, each computes 32 output
rows of one batch element.

Per core pipeline (4 groups of 1024 positions):
 1. One fused 3x3 conv (PE, f32) produces 45 rows: [off_x(9) | off_y(9) |
    ad(9, replicated n%3) | ad(9, copy) | m(9)] per position.
 2. f32 coordinate math (DVE/ACT) reproduces the reference's floor/clamp/mask
    bilinear weights exactly; sampling indices r1 = clamp(floor(p)+1, 0, 130)
    into a host-built edge-replicated "pair table".
 3. dma_gather (transpose mode, bf16) fetches 256B tokens = 2 adjacent pixels
    x 64 channels, landing channel-major: [128=(px,c), tokens].
 4. One elementwise multiply per gathered element applies mm*gx*gy (bf16).
 5. The 4-corner bilinear sum AND the final 3x3 (stride-3) conv collapse into
    PE matmuls with a K=(n,corner_row,px,c)=2304 contraction using the conv
    weight replicated over the 4 corners (f32 PSUM accumulation).

The gather table, weight repacks, and coordinate base planes are prepared on
the host in numpy (layout/sharding prep only — no FLOPs on tensor data other
than the bf16 cast of the table).
"""

import numpy as np
import ml_dtypes

# ---- problem constants (hardcoded per contract) ----
B, C, H, W = 2, 64, 128, 128
KS, N, DIL, PAD = 3, 9, 2, 1
Hp = H + 2 * PAD            # 130
EXT = Hp + 2                # 132 (edge-replicated ext rows/cols)
NPIX = EXT * EXT            # 17424
NCORES = 8
HSH = H // 4                # 32 rows per core
NPOS = HSH * W              # 4096 positions per core
NG = 4                      # groups per core
GPOS = NPOS // NG           # 1024 positions per group
NBLK = GPOS // 128          # 8 pos-blocks of 128 per group
NTOK = GPOS * N * 2         # 18432 gather tokens per group
M_CONV = 73                 # fused conv rows: off 0:18 | ad 32:50 | m 64:73

_cache = {}


# ======================================================================
# host-side input preparation
# ======================================================================

def _prep_consts(w_p, b_p, w_m, w_ad, w_conv):
    f32 = np.float32
    # fused conv taps: wt[t, c, m], t = dy*3+dx
    wt = np.zeros((9, C, M_CONV), f32)
    rep3 = [0, 1, 2, 0, 1, 2, 0, 1, 2]
    for t in range(9):
        dy, dx = t // 3, t % 3
        wt[t, :, 0:9] = w_p[0:9, :, dy, dx].T
        wt[t, :, 9:18] = w_p[9:18, :, dy, dx].T
        wt[t, :, 32:41] = w_ad[rep3, :, dy, dx].T
        wt[t, :, 41:50] = w_ad[rep3, :, dy, dx].T
        wt[t, :, 64:73] = w_m[0:9, :, dy, dx].T
    wt = np.ascontiguousarray(wt.transpose(1, 0, 2).reshape(C, 9 * M_CONV))

    bp2 = b_p.reshape(18, 1).astype(f32)

    r = np.array([-1.0, 0.0, 1.0], f32)
    pnx = np.repeat(r, 3)        # [-1,-1,-1,0,0,0,1,1,1]
    pny = np.tile(r, 3)          # [-1,0,1,-1,0,1,-1,0,1]
    pn18 = np.concatenate([pnx, pny]).reshape(18, 1).astype(f32)

    # idx matmul matrix: [18, 9]: idx1 = EXT*R1x + R1y
    lconst = np.zeros((18, 9), f32)
    for n in range(9):
        lconst[n, n] = EXT
        lconst[9 + n, n] = 1.0
    # w3: [128=(px,c), 18*64] bf16, k-tile t=(n*2+r) cols t*64:(t+1)*64
    w3 = np.zeros((128, 18 * 64), f32)
    for t in range(18):
        n = t % 9
        blk = w_conv[:, :, n // 3, n % 3].T  # [c, o]
        w3[0:64, t * 64:(t + 1) * 64] = blk
        w3[64:128, t * 64:(t + 1) * 64] = blk
    w3 = w3.astype(ml_dtypes.bfloat16)

    ident = np.eye(128, dtype=f32)
    return dict(wt=wt, bp2=bp2, pn18=pn18, lconst=lconst, w3=w3, ident=ident)


def _prep_table(xb):
    """xb: [C, H, W] f32 -> pair table [NPIX, 128] bf16."""
    xp = np.pad(xb, ((0, 0), (PAD, PAD), (PAD, PAD)))          # [C, 130, 130]
    idx = np.clip(np.arange(EXT) - 1, 0, Hp - 1)
    ext = xp[:, idx][:, :, idx]                                # [C, 132, 132]
    flat = np.ascontiguousarray(ext.transpose(1, 2, 0)).reshape(NPIX, C)
    nxt = np.concatenate([flat[1:], flat[-1:]], axis=0)
    pair = np.concatenate([flat, nxt], axis=1)                 # [NPIX, 128]
    return pair.astype(ml_dtypes.bfloat16)


def _prep_core_inputs(core, x, consts):
    b, hc = core // 4, core % 4
    h0 = hc * HSH
    # conv input rows h0-1 .. h0+32 (34 rows), zero padded at batch edges
    xs = np.zeros((C, HSH + 2, W), np.float32)
    lo, hi = h0 - 1, h0 + HSH + 1
    slo, shi = max(lo, 0), min(hi, H)
    xs[:, slo - lo:shi - lo, :] = x[b, :, slo:shi, :]

    base = np.zeros((18, NPOS), np.float32)
    pos = np.arange(NPOS)
    base[0:9, :] = (h0 + pos // W + 1)[None, :]
    base[9:18, :] = (pos % W + 1)[None, :]

    m = dict(xs=xs, base18=base, xe=_cache[('xe', b)])
    m.update({k: consts[k] for k in ('wt', 'bp2', 'pn18', 'lconst', 'w3', 'ident')})
    return m


# ======================================================================
# bass program
# ======================================================================

def _emit(nc, tc, t):
    import concourse.bass as bass
    import concourse.mybir as mybir
    from concourse.bass import AP

    dt = mybir.dt
    ALU = mybir.AluOpType
    ACTF = mybir.ActivationFunctionType
    f32, bf16, i16, i32 = dt.float32, dt.bfloat16, dt.int16, dt.int32

    XROW = HSH + 2          # 34
    XCW = W + 2             # 130 padded row width in sbuf
    any_, vec, act, pe, gp, snc = nc.any, nc.vector, nc.scalar, nc.tensor, nc.gpsimd, nc.sync

    HTOK = NTOK // 2        # 9216 tokens per half-group gather
    with tc.tile_pool(name="const", bufs=1) as cpool, \
         tc.tile_pool(name="work", bufs=1) as wpool, \
         tc.tile_pool(name="gath", bufs=2) as gpool, \
         tc.tile_pool(name="gath2", bufs=2) as g2pool, \
         tc.tile_pool(name="psA", bufs=2, space="PSUM") as psA, \
         tc.tile_pool(name="psB", bufs=2, space="PSUM") as psB:

        # ---- load constants / inputs to SBUF ----
        xsb = cpool.tile([C, XROW * XCW], f32, tag="xsb")
        gp.memset(xsb[:], 0.0)
        snc.dma_start(
            out=AP(xsb.tensor, xsb[:].offset + 1,
                   [[XROW * XCW, C], [XCW, XROW], [1, W]]),
            in_=t['xs'][:])
        wtb = cpool.tile([C, 9 * M_CONV], f32, tag="wtb")
        snc.dma_start(out=wtb[:], in_=t['wt'][:])
        w3b = cpool.tile([128, 18 * 64], bf16, tag="w3b")
        snc.dma_start(out=w3b[:], in_=t['w3'][:])
        idb = cpool.tile([18, 18], f32, tag="idb")
        snc.dma_start(out=idb[:], in_=t['ident'][:18, :18])
        bpb = cpool.tile([18, 1], f32, tag="bpb")
        snc.dma_start(out=bpb[:], in_=t['bp2'][:])
        pnb = cpool.tile([18, 1], f32, tag="pnb")
        snc.dma_start(out=pnb[:], in_=t['pn18'][:])
        lcb = cpool.tile([18, 9], f32, tag="lcb")
        snc.dma_start(out=lcb[:], in_=t['lconst'][:])
        mbias = cpool.tile([18, 1], f32, tag="mbias")
        gp.memset(mbias[:], -64.5)

        xe = t['xe']  # dram [NPIX, 128] bf16

        for g in range(NG):
            # ---------------- fused conv ----------------
            conv = wpool.tile([M_CONV, GPOS], f32, tag="conv")
            for k in range(2):
                pc = psA.tile([M_CONV, 512], f32, tag="pc")
                for tap in range(9):
                    dy, dx = tap // 3, tap % 3
                    rhs = AP(xsb.tensor,
                             xsb[:].offset + (g * 8 + k * 4 + dy) * XCW + dx,
                             [[XROW * XCW, C], [XCW, 4], [1, W]])
                    pe.matmul(pc[:], wtb[:, tap * M_CONV:(tap + 1) * M_CONV],
                              rhs, start=(tap == 0), stop=(tap == 8))
                sl = slice(k * 512, (k + 1) * 512)
                # rows 0:18 offsets + bias; 18:36 -> 1-sigmoid; 36:45 -> sigmoid
                act.activation(conv[0:18, sl], pc[0:18, :], ACTF.Identity,
                               bias=bpb[:], scale=1.0)
                act.activation(conv[32:50, sl], pc[32:50, :], ACTF.Sigmoid,
                               scale=-1.0)
                act.activation(conv[64:73, sl], pc[64:73, :], ACTF.Sigmoid,
                               scale=1.0)

            OFF, SGN, SGM = conv[0:18, :], conv[32:50, :], conv[64:73, :]
            bsl = wpool.tile([18, GPOS], f32, tag="bsl")
            snc.dma_start(out=bsl[:], in_=t['base18'][:, g * GPOS:(g + 1) * GPOS])

            # ---------------- coordinates (f32) ----------------
            # ADF = 1 + 2*(1-sigmoid) written over the SGN rows in place
            vec.tensor_scalar(SGN, SGN, 2.0, 1.0, ALU.mult, ALU.add)
            # base-0 copies (walrus: TensorScalarPtr/TensorTensor SBUF inputs
            # must share base partition; DMA is exempt)
            ADF = wpool.tile([18, GPOS], f32, tag="adf0")
            snc.dma_start(out=ADF[:], in_=SGN)
            SGM0 = wpool.tile([9, GPOS], f32, tag="sgm0")
            snc.dma_start(out=SGM0[:], in_=SGM)
            V = wpool.tile([18, GPOS], f32, tag="v")
            vec.tensor_tensor(V[:], OFF, bsl[:], op=ALU.add)
            vec.scalar_tensor_tensor(V[:], ADF[:], pnb[:], V[:],
                                     op0=ALU.mult, op1=ALU.add)
            # floor(V) robust to convert rounding mode: g=int(V); F=g-(g>V)
            I32T = wpool.tile([18, GPOS], i32, tag="i32t")
            vec.tensor_copy(I32T[:], V[:])
            F = wpool.tile([18, GPOS], f32, tag="f")
            vec.tensor_copy(F[:], I32T[:])
            FRAC = wpool.tile([18, GPOS], f32, tag="frac")
            vec.tensor_tensor(FRAC[:], F[:], V[:], op=ALU.is_gt)
            vec.tensor_tensor(F[:], F[:], FRAC[:], op=ALU.subtract)
            vec.tensor_tensor(FRAC[:], V[:], F[:], op=ALU.subtract)
            QRB1 = wpool.tile([18, GPOS], f32, tag="qrb1")
            vec.tensor_scalar(QRB1[:], F[:], 1.0, 0.0, ALU.add, ALU.max)
            # QLT over F in place
            vec.tensor_scalar(F[:], F[:], 0.0, Hp - 1.0, ALU.max, ALU.min)
            QLT = F
            R1 = wpool.tile([18, GPOS], f32, tag="r1")
            vec.tensor_scalar(R1[:], QRB1[:], Hp + 0.0, None, ALU.min)
            # QRB over QRB1 in place (after R1 consumed the pre-min value)
            vec.tensor_scalar(QRB1[:], QRB1[:], Hp - 1.0, None, ALU.min)
            QRB = QRB1
            MASK = wpool.tile([18, GPOS], f32, tag="mask")
            act.activation(MASK[:], V[:], ACTF.Abs, bias=mbias[:], scale=1.0)
            vec.tensor_scalar(MASK[:], MASK[:], 63.5, None, ALU.is_gt)
            vec.tensor_tensor(MASK[:], MASK[:], FRAC[:], op=ALU.mult)
            # PF = clamp(V - mask*frac) stored into V
            vec.tensor_tensor(V[:], V[:], MASK[:], op=ALU.subtract)
            vec.tensor_scalar(V[:], V[:], 0.0, Hp - 1.0, ALU.max, ALU.min)
            GLT = wpool.tile([18, GPOS], f32, tag="glt")
            vec.scalar_tensor_tensor(GLT[:], QLT[:], 1.0, V[:],
                                     op0=ALU.add, op1=ALU.subtract)
            GRB = wpool.tile([18, GPOS], f32, tag="grb")
            vec.scalar_tensor_tensor(GRB[:], V[:], 1.0, QRB[:],
                                     op0=ALU.add, op1=ALU.subtract)

            # ---------------- modulation & corner weights (bf16 out) ----
            ADM = wpool.tile([9, GPOS], f32, tag="adm")
            vec.tensor_scalar(ADM[:], ADF[0:9, :], 2.0, -4.0, ALU.mult, ALU.add)
            vec.tensor_tensor(ADM[:], SGM0[:], ADM[:], op=ALU.mult)  # ADM = mm
            T1 = wpool.tile([9, GPOS], f32, tag="t1")
            vec.tensor_tensor(T1[:], ADM[:], GLT[0:9, :], op=ALU.mult)
            T2 = wpool.tile([9, GPOS], f32, tag="t2")
            vec.tensor_tensor(T2[:], ADM[:], GRB[0:9, :], op=ALU.mult)
            # y-halves moved to partition-0-based tiles (engines need aligned
            # partition starts; DMA is exempt from that rule)
            GLTY = wpool.tile([9, GPOS], f32, tag="glty")
            snc.dma_start(out=GLTY[:], in_=GLT[9:18, :])
            GRBY = wpool.tile([9, GPOS], f32, tag="grby")
            snc.dma_start(out=GRBY[:], in_=GRB[9:18, :])

            # ---------------- indices ----------------
            idxf = wpool.tile([9, GPOS], f32, tag="idxf")
            for k in range(2):
                pi = psB.tile([9, 512], f32, tag="pi")
                pe.matmul(pi[:], lcb[:], R1[:, k * 512:(k + 1) * 512],
                          start=True, stop=True)
                # permuted store: col bl*128 + q*8 + s <- pi col bl*128+s*16+q
                act.activation(
                    AP(idxf.tensor, idxf[:].offset + k * 512,
                       [[GPOS, 9], [128, 4], [1, 8], [8, 16]]),
                    pi[:], ACTF.Copy, scale=1.0)
            idxt = wpool.tile([128, NBLK * 18], i16, tag="idxt")
            for bl in range(NBLK):
                tp = psB.tile([128, 9], f32, tag="tp")
                pe.transpose(tp[:], idxf[:, bl * 128:(bl + 1) * 128],
                             idb[0:9, 0:9])
                vec.tensor_copy(idxt[:, bl * 18:bl * 18 + 9], tp[:])
                vec.tensor_scalar(idxt[:, bl * 18 + 9:bl * 18 + 18], tp[:],
                                  float(EXT), None, ALU.add)

            for h in range(2):
                # corner weights for this half-group (bf16), [9, 512] each
                sl5 = slice(h * 512, (h + 1) * 512)
                WC = []
                for (srcw, gy, tag) in ((T1, GLTY, "w00"), (T1, GRBY, "w01"),
                                        (T2, GLTY, "w10"), (T2, GRBY, "w11")):
                    wc = wpool.tile([9, 512], bf16, tag=tag)
                    vec.tensor_tensor(wc[:], srcw[0:9, sl5], gy[0:9, sl5],
                                      op=ALU.mult)
                    WC.append(wc)

                # shuffle [128,(b,nr)] -> wrapped [16, (b*8+s)*18+nr]
                idxw = wpool.tile([16, HTOK // 16], i16, tag="idxw")
                for bl in range(4):
                    snc.dma_start(
                        out=AP(idxw.tensor, idxw[:].offset + bl * 144,
                               [[HTOK // 16, 16], [18, 8], [1, 18]]),
                        in_=AP(idxt.tensor,
                               idxt[:].offset + (h * 4 + bl) * 18,
                               [[NBLK * 18, 128], [1, 18]]))
                idxr = wpool.tile([128, HTOK // 16], i16, tag="idxr")
                snc.dma_start(out=idxr[0:16, :], in_=idxw[:])
                snc.dma_start(out=idxr[16:32, :], in_=idxr[0:16, :])
                snc.dma_start(out=idxr[32:64, :], in_=idxr[0:32, :])
                snc.dma_start(out=idxr[64:128, :], in_=idxr[0:64, :])

                # ---------------- gather ----------------
                G = gpool.tile([128, HTOK], bf16, tag="G")
                gp.dma_gather(
                    out_ap=AP(G.tensor, G[:].offset,
                              [[HTOK, 128], [HTOK, 1], [1, HTOK]]),
                    in_ap=xe[:],
                    idxs_ap=idxr[:],
                    num_idxs=HTOK,
                    num_idxs_reg=HTOK,
                    elem_size=128,
                    elem_step=128,
                    transpose=True,
                    single_packet=False)

                # ---------------- replicate weights across partitions ----
                # WRB[p, r*4608 + n*512 + pos] = w_{n, r, pp=p//64}[pos]
                WRB = wpool.tile([128, 2 * 4608], bf16, tag="wrb")
                PIT = 2 * 4608
                for r in range(2):
                    for pp in range(2):
                        # seed: wc [9, 512] -> one flat row of WRB
                        snc.dma_start(
                            out=AP(WRB.tensor,
                                   WRB[:].offset + pp * 64 * PIT + r * 4608,
                                   [[PIT, 1], [1, 4608]]),
                            in_=WC[r * 2 + pp][:])
                # doubling across partitions
                for kk in (1, 2, 4, 8, 16, 32):
                    for pp in range(2):
                        sl_src = AP(WRB.tensor,
                                    WRB[:].offset + pp * 64 * PIT,
                                    [[PIT, kk], [1, PIT]])
                        sl_dst = AP(WRB.tensor,
                                    WRB[:].offset + (pp * 64 + kk) * PIT,
                                    [[PIT, kk], [1, PIT]])
                        snc.dma_start(out=sl_dst, in_=sl_src)

                # ---------------- apply weights ----------------
                G2 = g2pool.tile([128, HTOK], bf16, tag="G2")
                for n in range(9):
                    for r in range(2):
                        j = r * 9 + n
                        sap = [[HTOK, 128], [2304, 4], [288, 8], [1, 16]]
                        src = AP(G.tensor, G[:].offset + j * 16, sap)
                        dst = AP(G2.tensor, G2[:].offset + j * 16, sap)
                        wap = AP(WRB.tensor,
                                 WRB[:].offset + r * 4608 + n * 512,
                                 [[2 * 4608, 128], [128, 4], [16, 8], [1, 16]])
                        any_.tensor_tensor(dst, src, wap, op=ALU.mult)

                # ---------------- final matmuls ----------------
                po = psA.tile([64, 512], f32, tag="po")
                for tt in range(18):
                    rhs = AP(G2.tensor, G2[:].offset + tt * 16,
                             [[HTOK, 128], [2304, 4], [288, 8], [1, 16]])
                    pe.matmul(po[:], w3b[:, tt * 64:(tt + 1) * 64], rhs,
                              start=(tt == 0), stop=(tt == 17))
                oc = wpool.tile([64, 512], f32, tag="oc")
                act.activation(oc[:], po[:], ACTF.Copy, scale=1.0)
                snc.dma_start(
                    out=t['outp'][:, g * GPOS + h * 512:g * GPOS + (h + 1) * 512],
                    in_=oc[:])


def _build():
    import concourse.bacc as bacc
    import concourse.tile as tile
    import concourse.mybir as mybir
    dt = mybir.dt

    nc = bacc.Bacc("TRN2", target_bir_lowering=False, debug=False)
    t = {}
    specs = [
        ('xs', [C, HSH + 2, W], dt.float32),
        ('xe', [NPIX, 128], dt.bfloat16),
        ('wt', [C, 9 * M_CONV], dt.float32),
        ('bp2', [18, 1], dt.float32),
        ('pn18', [18, 1], dt.float32),
        ('lconst', [18, 9], dt.float32),
        ('w3', [128, 18 * 64], dt.bfloat16),
        ('ident', [128, 128], dt.float32),
        ('base18', [18, NPOS], dt.float32),
    ]
    for name, shape, d in specs:
        t[name] = nc.dram_tensor(name, shape, d, kind="ExternalInput").ap()
    t['outp'] = nc.dram_tensor('outp', [64, NPOS], dt.float32,
                               kind="ExternalOutput").ap()
    with tile.TileContext(nc) as tc:
        _emit(nc, tc, t)
    nc.compile()
    return nc


def kernel(x, w_p, b_p, w_m, w_ad, w_conv):
    from concourse.bass_utils import run_bass_kernel_spmd

    x = np.asarray(x, np.float32)
    consts = _prep_consts(np.asarray(w_p, np.float32), np.asarray(b_p, np.float32),
                          np.asarray(w_m, np.float32), np.asarray(w_ad, np.float32),
                          np.asarray(w_conv, np.float32))
    for b in range(B):
        _cache[('xe', b)] = _prep_table(x[b])
    if 'nc' not in _cache:
        _cache['nc'] = _build()
    nc = _cache['nc']

    in_maps = [_prep_core_inputs(c, x, consts) for c in range(NCORES)]
    res = run_bass_kernel_spmd(nc, in_maps, list(range(NCORES)))
    _cache['last_results'] = res

    out = np.zeros((B, 64, H, W), np.float32)
    for c in range(NCORES):
        b, hc = c // 4, c % 4
        out[b, :, hc * HSH:(hc + 1) * HSH, :] = \
            res.results[c]['outp'].reshape(64, HSH, W)
    return out



# revision 4
# speedup vs baseline: 1.0307x; 1.0307x over previous
"""Deformable Conv2d (adaptive, modulated) for Trainium2 — 8-core SPMD Bass kernel.

Strategy
--------
Shard (batch, H) into 8 shards: core = b*4 + hchunk, each computes 32 output
rows of one batch element.

Per core pipeline (4 groups of 1024 positions):
 1. One fused 3x3 conv (PE, f32) produces 45 rows: [off_x(9) | off_y(9) |
    ad(9, replicated n%3) | ad(9, copy) | m(9)] per position.
 2. f32 coordinate math (DVE/ACT) reproduces the reference's floor/clamp/mask
    bilinear weights exactly; sampling indices r1 = clamp(floor(p)+1, 0, 130)
    into a host-built edge-replicated "pair table".
 3. dma_gather (transpose mode, bf16) fetches 256B tokens = 2 adjacent pixels
    x 64 channels, landing channel-major: [128=(px,c), tokens].
 4. One elementwise multiply per gathered element applies mm*gx*gy (bf16).
 5. The 4-corner bilinear sum AND the final 3x3 (stride-3) conv collapse into
    PE matmuls with a K=(n,corner_row,px,c)=2304 contraction using the conv
    weight replicated over the 4 corners (f32 PSUM accumulation).

The gather table, weight repacks, and coordinate base planes are prepared on
the host in numpy (layout/sharding prep only — no FLOPs on tensor data other
than the bf16 cast of the table).
"""

import numpy as np
import ml_dtypes

# ---- problem constants (hardcoded per contract) ----
B, C, H, W = 2, 64, 128, 128
KS, N, DIL, PAD = 3, 9, 2, 1
Hp = H + 2 * PAD            # 130
EXT = Hp + 2                # 132 (edge-replicated ext rows/cols)
NPIX = EXT * EXT            # 17424
NCORES = 8
HSH = H // 4                # 32 rows per core
NPOS = HSH * W              # 4096 positions per core
NG = 4                      # groups per core
GPOS = NPOS // NG           # 1024 positions per group
NBLK = GPOS // 128          # 8 pos-blocks of 128 per group
NTOK = GPOS * N * 2         # 18432 gather tokens per group
M_CONV = 73                 # fused conv rows: off 0:18 | ad 32:50 | m 64:73

_cache = {}


# ======================================================================
# host-side input preparation
# ======================================================================

def _prep_consts(w_p, b_p, w_m, w_ad, w_conv):
    f32 = np.float32
    # fused conv taps: wt[t, c, m], t = dy*3+dx
    wt = np.zeros((9, C, M_CONV), f32)
    rep3 = [0, 1, 2, 0, 1, 2, 0, 1, 2]
    for t in range(9):
        dy, dx = t // 3, t % 3
        wt[t, :, 0:9] = w_p[0:9, :, dy, dx].T
        wt[t, :, 9:18] = w_p[9:18, :, dy, dx].T
        wt[t, :, 32:41] = w_ad[rep3, :, dy, dx].T
        wt[t, :, 41:50] = w_ad[rep3, :, dy, dx].T
        wt[t, :, 64:73] = w_m[0:9, :, dy, dx].T
    wt = np.ascontiguousarray(wt.transpose(1, 0, 2).reshape(C, 9 * M_CONV))

    bp2 = b_p.reshape(18, 1).astype(f32)

    r = np.array([-1.0, 0.0, 1.0], f32)
    pnx = np.repeat(r, 3)        # [-1,-1,-1,0,0,0,1,1,1]
    pny = np.tile(r, 3)          # [-1,0,1,-1,0,1,-1,0,1]
    pn18 = np.concatenate([pnx, pny]).reshape(18, 1).astype(f32)

    # idx matmul matrix: [18, 9]: idx1 = EXT*R1x + R1y
    lconst = np.zeros((18, 9), f32)
    for n in range(9):
        lconst[n, n] = EXT
        lconst[9 + n, n] = 1.0
    # w3: [128=(px,c), 18*64] bf16, k-tile t=(n*2+r) cols t*64:(t+1)*64
    w3 = np.zeros((128, 18 * 64), f32)
    for t in range(18):
        n = t % 9
        blk = w_conv[:, :, n // 3, n % 3].T  # [c, o]
        w3[0:64, t * 64:(t + 1) * 64] = blk
        w3[64:128, t * 64:(t + 1) * 64] = blk
    w3 = w3.astype(ml_dtypes.bfloat16)

    ident = np.eye(128, dtype=f32)
    return dict(wt=wt, bp2=bp2, pn18=pn18, lconst=lconst, w3=w3, ident=ident)


def _prep_table(xb):
    """xb: [C, H, W] f32 -> pair table [NPIX, 128] bf16."""
    xp = np.pad(xb, ((0, 0), (PAD, PAD), (PAD, PAD)))          # [C, 130, 130]
    idx = np.clip(np.arange(EXT) - 1, 0, Hp - 1)
    ext = xp[:, idx][:, :, idx]                                # [C, 132, 132]
    flat = np.ascontiguousarray(ext.transpose(1, 2, 0)).reshape(NPIX, C)
    nxt = np.concatenate([flat[1:], flat[-1:]], axis=0)
    pair = np.concatenate([flat, nxt], axis=1)                 # [NPIX, 128]
    return pair.astype(ml_dtypes.bfloat16)


def _prep_core_inputs(core, x, consts):
    b, hc = core // 4, core % 4
    h0 = hc * HSH
    # conv input rows h0-1 .. h0+32 (34 rows), zero padded at batch edges
    xs = np.zeros((C, HSH + 2, W), np.float32)
    lo, hi = h0 - 1, h0 + HSH + 1
    slo, shi = max(lo, 0), min(hi, H)
    xs[:, slo - lo:shi - lo, :] = x[b, :, slo:shi, :]

    base = np.zeros((18, NPOS), np.float32)
    pos = np.arange(NPOS)
    base[0:9, :] = (h0 + pos // W + 1)[None, :]
    base[9:18, :] = (pos % W + 1)[None, :]

    m = dict(xs=xs, base18=base, xe=_cache[('xe', b)])
    m.update({k: consts[k] for k in ('wt', 'bp2', 'pn18', 'lconst', 'w3', 'ident')})
    return m


# ======================================================================
# bass program
# ======================================================================

def _emit(nc, tc, t):
    import concourse.bass as bass
    import concourse.mybir as mybir
    from concourse.bass import AP

    dt = mybir.dt
    ALU = mybir.AluOpType
    ACTF = mybir.ActivationFunctionType
    f32, bf16, i16, i32 = dt.float32, dt.bfloat16, dt.int16, dt.int32

    XROW = HSH + 2          # 34
    XCW = W + 2             # 130 padded row width in sbuf
    any_, vec, act, pe, gp, snc = nc.any, nc.vector, nc.scalar, nc.tensor, nc.gpsimd, nc.sync

    HTOK = NTOK // 2        # 9216 tokens per half-group gather
    with tc.tile_pool(name="const", bufs=1) as cpool, \
         tc.tile_pool(name="work", bufs=1) as wpool, \
         tc.tile_pool(name="gath", bufs=2) as gpool, \
         tc.tile_pool(name="gath2", bufs=2) as g2pool, \
         tc.tile_pool(name="psA", bufs=2, space="PSUM") as psA, \
         tc.tile_pool(name="psB", bufs=2, space="PSUM") as psB:

        # ---- load constants / inputs to SBUF ----
        xsb = cpool.tile([C, XROW * XCW], f32, tag="xsb")
        gp.memset(xsb[:], 0.0)
        snc.dma_start(
            out=AP(xsb.tensor, xsb[:].offset + 1,
                   [[XROW * XCW, C], [XCW, XROW], [1, W]]),
            in_=t['xs'][:])
        wtb = cpool.tile([C, 9 * M_CONV], f32, tag="wtb")
        snc.dma_start(out=wtb[:], in_=t['wt'][:])
        w3b = cpool.tile([128, 18 * 64], bf16, tag="w3b")
        snc.dma_start(out=w3b[:], in_=t['w3'][:])
        idb = cpool.tile([18, 18], f32, tag="idb")
        snc.dma_start(out=idb[:], in_=t['ident'][:18, :18])
        bpb = cpool.tile([18, 1], f32, tag="bpb")
        snc.dma_start(out=bpb[:], in_=t['bp2'][:])
        pnb = cpool.tile([18, 1], f32, tag="pnb")
        snc.dma_start(out=pnb[:], in_=t['pn18'][:])
        lcb = cpool.tile([18, 9], f32, tag="lcb")
        snc.dma_start(out=lcb[:], in_=t['lconst'][:])
        mbias = cpool.tile([18, 1], f32, tag="mbias")
        gp.memset(mbias[:], -64.5)

        xe = t['xe']  # dram [NPIX, 128] bf16

        for g in range(NG):
            # ---------------- fused conv ----------------
            conv = wpool.tile([M_CONV, GPOS], f32, tag="conv")
            for k in range(2):
                pc = psA.tile([M_CONV, 512], f32, tag="pc")
                for tap in range(9):
                    dy, dx = tap // 3, tap % 3
                    rhs = AP(xsb.tensor,
                             xsb[:].offset + (g * 8 + k * 4 + dy) * XCW + dx,
                             [[XROW * XCW, C], [XCW, 4], [1, W]])
                    pe.matmul(pc[:], wtb[:, tap * M_CONV:(tap + 1) * M_CONV],
                              rhs, start=(tap == 0), stop=(tap == 8))
                sl = slice(k * 512, (k + 1) * 512)
                # rows 0:18 offsets + bias; 18:36 -> 1-sigmoid; 36:45 -> sigmoid
                act.activation(conv[0:18, sl], pc[0:18, :], ACTF.Identity,
                               bias=bpb[:], scale=1.0)
                act.activation(conv[32:50, sl], pc[32:50, :], ACTF.Sigmoid,
                               scale=-1.0)
                act.activation(conv[64:73, sl], pc[64:73, :], ACTF.Sigmoid,
                               scale=1.0)

            OFF, SGN, SGM = conv[0:18, :], conv[32:50, :], conv[64:73, :]
            bsl = wpool.tile([18, GPOS], f32, tag="bsl")
            snc.dma_start(out=bsl[:], in_=t['base18'][:, g * GPOS:(g + 1) * GPOS])

            # ---------------- coordinates (f32) ----------------
            # ADF = 1 + 2*(1-sigmoid) written over the SGN rows in place
            vec.tensor_scalar(SGN, SGN, 2.0, 1.0, ALU.mult, ALU.add)
            # base-0 copies (walrus: TensorScalarPtr/TensorTensor SBUF inputs
            # must share base partition; DMA is exempt)
            ADF = wpool.tile([18, GPOS], f32, tag="adf0")
            snc.dma_start(out=ADF[:], in_=SGN)
            SGM0 = wpool.tile([9, GPOS], f32, tag="sgm0")
            snc.dma_start(out=SGM0[:], in_=SGM)
            V = wpool.tile([18, GPOS], f32, tag="v")
            vec.tensor_tensor(V[:], OFF, bsl[:], op=ALU.add)
            vec.scalar_tensor_tensor(V[:], ADF[:], pnb[:], V[:],
                                     op0=ALU.mult, op1=ALU.add)
            # floor(V) robust to convert rounding mode: g=int(V); F=g-(g>V)
            I32T = wpool.tile([18, GPOS], i32, tag="i32t")
            vec.tensor_copy(I32T[:], V[:])
            F = wpool.tile([18, GPOS], f32, tag="f")
            vec.tensor_copy(F[:], I32T[:])
            FRAC = wpool.tile([18, GPOS], f32, tag="frac")
            vec.tensor_tensor(FRAC[:], F[:], V[:], op=ALU.is_gt)
            vec.tensor_tensor(F[:], F[:], FRAC[:], op=ALU.subtract)
            vec.tensor_tensor(FRAC[:], V[:], F[:], op=ALU.subtract)
            QRB1 = wpool.tile([18, GPOS], f32, tag="qrb1")
            vec.tensor_scalar(QRB1[:], F[:], 1.0, 0.0, ALU.add, ALU.max)
            # QLT over F in place
            vec.tensor_scalar(F[:], F[:], 0.0, Hp - 1.0, ALU.max, ALU.min)
            QLT = F
            R1 = wpool.tile([18, GPOS], f32, tag="r1")
            vec.tensor_scalar(R1[:], QRB1[:], Hp + 0.0, None, ALU.min)
            # QRB over QRB1 in place (after R1 consumed the pre-min value)
            vec.tensor_scalar(QRB1[:], QRB1[:], Hp - 1.0, None, ALU.min)
            QRB = QRB1
            MASK = wpool.tile([18, GPOS], f32, tag="mask")
            act.activation(MASK[:], V[:], ACTF.Abs, bias=mbias[:], scale=1.0)
            vec.tensor_scalar(MASK[:], MASK[:], 63.5, None, ALU.is_gt)
            vec.tensor_tensor(MASK[:], MASK[:], FRAC[:], op=ALU.mult)
            # PF = clamp(V - mask*frac) stored into V
            vec.tensor_tensor(V[:], V[:], MASK[:], op=ALU.subtract)
            vec.tensor_scalar(V[:], V[:], 0.0, Hp - 1.0, ALU.max, ALU.min)
            GLT = wpool.tile([18, GPOS], f32, tag="glt")
            vec.scalar_tensor_tensor(GLT[:], QLT[:], 1.0, V[:],
                                     op0=ALU.add, op1=ALU.subtract)
            GRB = wpool.tile([18, GPOS], f32, tag="grb")
            vec.scalar_tensor_tensor(GRB[:], V[:], 1.0, QRB[:],
                                     op0=ALU.add, op1=ALU.subtract)

            # ---------------- modulation & corner weights (bf16 out) ----
            ADM = wpool.tile([9, GPOS], f32, tag="adm")
            vec.tensor_scalar(ADM[:], ADF[0:9, :], 2.0, -4.0, ALU.mult, ALU.add)
            vec.tensor_tensor(ADM[:], SGM0[:], ADM[:], op=ALU.mult)  # ADM = mm
            T1 = wpool.tile([9, GPOS], f32, tag="t1")
            vec.tensor_tensor(T1[:], ADM[:], GLT[0:9, :], op=ALU.mult)
            T2 = wpool.tile([9, GPOS], f32, tag="t2")
            vec.tensor_tensor(T2[:], ADM[:], GRB[0:9, :], op=ALU.mult)
            # y-halves moved to partition-0-based tiles (engines need aligned
            # partition starts; DMA is exempt from that rule)
            GLTY = wpool.tile([9, GPOS], f32, tag="glty")
            snc.dma_start(out=GLTY[:], in_=GLT[9:18, :])
            GRBY = wpool.tile([9, GPOS], f32, tag="grby")
            snc.dma_start(out=GRBY[:], in_=GRB[9:18, :])

            # ---------------- indices ----------------
            idxf = wpool.tile([9, GPOS], f32, tag="idxf")
            for k in range(2):
                pi = psB.tile([9, 512], f32, tag="pi")
                pe.matmul(pi[:], lcb[:], R1[:, k * 512:(k + 1) * 512],
                          start=True, stop=True)
                # permuted store: col bl*128 + q*8 + s <- pi col bl*128+s*16+q
                act.activation(
                    AP(idxf.tensor, idxf[:].offset + k * 512,
                       [[GPOS, 9], [128, 4], [1, 8], [8, 16]]),
                    pi[:], ACTF.Copy, scale=1.0)
            idxt = wpool.tile([128, NBLK * 18], i16, tag="idxt")
            for bl in range(NBLK):
                tp = psB.tile([128, 9], f32, tag="tp")
                pe.transpose(tp[:], idxf[:, bl * 128:(bl + 1) * 128],
                             idb[0:9, 0:9])
                vec.tensor_copy(idxt[:, bl * 18:bl * 18 + 9], tp[:])
                vec.tensor_scalar(idxt[:, bl * 18 + 9:bl * 18 + 18], tp[:],
                                  float(EXT), None, ALU.add)

            for h in range(2):
                # corner weights for this half-group (bf16), [9, 512] each
                sl5 = slice(h * 512, (h + 1) * 512)
                WC = []
                for (srcw, gy, tag) in ((T1, GLTY, "w00"), (T1, GRBY, "w01"),
                                        (T2, GLTY, "w10"), (T2, GRBY, "w11")):
                    wc = wpool.tile([9, 512], bf16, tag=tag)
                    vec.tensor_tensor(wc[:], srcw[0:9, sl5], gy[0:9, sl5],
                                      op=ALU.mult)
                    WC.append(wc)

                # shuffle [128,(b,nr)] -> wrapped [16, (b*8+s)*18+nr]
                idxw = wpool.tile([16, HTOK // 16], i16, tag="idxw")
                for bl in range(4):
                    snc.dma_start(
                        out=AP(idxw.tensor, idxw[:].offset + bl * 144,
                               [[HTOK // 16, 16], [18, 8], [1, 18]]),
                        in_=AP(idxt.tensor,
                               idxt[:].offset + (h * 4 + bl) * 18,
                               [[NBLK * 18, 128], [1, 18]]))
                idxr = wpool.tile([128, HTOK // 16], i16, tag="idxr")
                snc.dma_start(out=idxr[0:16, :], in_=idxw[:])
                snc.dma_start(out=idxr[16:32, :], in_=idxr[0:16, :])
                snc.dma_start(out=idxr[32:64, :], in_=idxr[0:32, :])
                snc.dma_start(out=idxr[64:128, :], in_=idxr[0:64, :])

                # ---------------- gather ----------------
                # Split into 2 column-half gathers on different SWDGE queues:
                # each queue runs on its own Q7 core pair, so with 2 half-
                # groups in flight all 4 queues generate descriptors
                # concurrently (desc-gen on one pair is the baseline
                # bottleneck at ~8 ns/desc).
                G = gpool.tile([128, HTOK], bf16, tag="G")
                QTOK = HTOK // 2          # 4608 idxs per queue-gather
                for qh in range(2):
                    qn = 0
                    gp.dma_gather(
                        out_ap=AP(G.tensor, G[:].offset + qh * QTOK,
                                  [[HTOK, 128], [HTOK, 1], [1, QTOK]]),
                        in_ap=xe[:],
                        idxs_ap=idxr[:, qh * (QTOK // 16):(qh + 1) * (QTOK // 16)],
                        num_idxs=QTOK,
                        num_idxs_reg=QTOK,
                        elem_size=128,
                        elem_step=128,
                        transpose=True,
                        single_packet=False,
                        queue_num=qn)

                # ---------------- replicate weights across partitions ----
                # WRB[p, r*4608 + n*512 + pos] = w_{n, r, pp=p//64}[pos]
                WRB = wpool.tile([128, 2 * 4608], bf16, tag="wrb")
                PIT = 2 * 4608
                for r in range(2):
                    for pp in range(2):
                        # seed: wc [9, 512] -> one flat row of WRB
                        snc.dma_start(
                            out=AP(WRB.tensor,
                                   WRB[:].offset + pp * 64 * PIT + r * 4608,
                                   [[PIT, 1], [1, 4608]]),
                            in_=WC[r * 2 + pp][:])
                # doubling across partitions
                for kk in (1, 2, 4, 8, 16, 32):
                    for pp in range(2):
                        sl_src = AP(WRB.tensor,
                                    WRB[:].offset + pp * 64 * PIT,
                                    [[PIT, kk], [1, PIT]])
                        sl_dst = AP(WRB.tensor,
                                    WRB[:].offset + (pp * 64 + kk) * PIT,
                                    [[PIT, kk], [1, PIT]])
                        snc.dma_start(out=sl_dst, in_=sl_src)

                # ---------------- apply weights ----------------
                G2 = g2pool.tile([128, HTOK], bf16, tag="G2")
                for n in range(9):
                    for r in range(2):
                        j = r * 9 + n
                        sap = [[HTOK, 128], [2304, 4], [288, 8], [1, 16]]
                        src = AP(G.tensor, G[:].offset + j * 16, sap)
                        dst = AP(G2.tensor, G2[:].offset + j * 16, sap)
                        wap = AP(WRB.tensor,
                                 WRB[:].offset + r * 4608 + n * 512,
                                 [[2 * 4608, 128], [128, 4], [16, 8], [1, 16]])
                        any_.tensor_tensor(dst, src, wap, op=ALU.mult)

                # ---------------- final matmuls ----------------
                po = psA.tile([64, 512], f32, tag="po")
                for tt in range(18):
                    rhs = AP(G2.tensor, G2[:].offset + tt * 16,
                             [[HTOK, 128], [2304, 4], [288, 8], [1, 16]])
                    pe.matmul(po[:], w3b[:, tt * 64:(tt + 1) * 64], rhs,
                              start=(tt == 0), stop=(tt == 17))
                oc = wpool.tile([64, 512], f32, tag="oc")
                act.activation(oc[:], po[:], ACTF.Copy, scale=1.0)
                snc.dma_start(
                    out=t['outp'][:, g * GPOS + h * 512:g * GPOS + (h + 1) * 512],
                    in_=oc[:])


def _build():
    import concourse.bacc as bacc
    import concourse.tile as tile
    import concourse.mybir as mybir
    dt = mybir.dt

    nc = bacc.Bacc("TRN2", target_bir_lowering=False, debug=False,
                   num_swdge_queues=4)
    t = {}
    specs = [
        ('xs', [C, HSH + 2, W], dt.float32),
        ('xe', [NPIX, 128], dt.bfloat16),
        ('wt', [C, 9 * M_CONV], dt.float32),
        ('bp2', [18, 1], dt.float32),
        ('pn18', [18, 1], dt.float32),
        ('lconst', [18, 9], dt.float32),
        ('w3', [128, 18 * 64], dt.bfloat16),
        ('ident', [128, 128], dt.float32),
        ('base18', [18, NPOS], dt.float32),
    ]
    for name, shape, d in specs:
        t[name] = nc.dram_tensor(name, shape, d, kind="ExternalInput").ap()
    t['outp'] = nc.dram_tensor('outp', [64, NPOS], dt.float32,
                               kind="ExternalOutput").ap()
    with tile.TileContext(nc) as tc:
        _emit(nc, tc, t)
    nc.compile()
    return nc


def kernel(x, w_p, b_p, w_m, w_ad, w_conv):
    from concourse.bass_utils import run_bass_kernel_spmd

    x = np.asarray(x, np.float32)
    consts = _prep_consts(np.asarray(w_p, np.float32), np.asarray(b_p, np.float32),
                          np.asarray(w_m, np.float32), np.asarray(w_ad, np.float32),
                          np.asarray(w_conv, np.float32))
    for b in range(B):
        _cache[('xe', b)] = _prep_table(x[b])
    if 'nc' not in _cache:
        _cache['nc'] = _build()
    nc = _cache['nc']

    in_maps = [_prep_core_inputs(c, x, consts) for c in range(NCORES)]
    res = run_bass_kernel_spmd(nc, in_maps, list(range(NCORES)))
    _cache['last_results'] = res

    out = np.zeros((B, 64, H, W), np.float32)
    for c in range(NCORES):
        b, hc = c // 4, c % 4
        out[b, :, hc * HSH:(hc + 1) * HSH, :] = \
            res.results[c]['outp'].reshape(64, HSH, W)
    return out



# revision 10
# speedup vs baseline: 1.1136x; 1.0804x over previous
"""Deformable Conv2d (adaptive, modulated) for Trainium2 — 8-core SPMD Bass kernel.

Strategy
--------
Shard (batch, H) into 8 shards: core = b*4 + hchunk, each computes 32 output
rows of one batch element.

Per core pipeline (4 groups of 1024 positions, each split in 2 half-groups):
 1. One fused 3x3 conv (PE, f32) produces 45 rows: [off_x(9) | off_y(9) |
    ad(9, replicated n%3) | ad(9, copy) | m(9)] per position.
 2. f32 coordinate math (DVE/ACT) reproduces the reference's floor/clamp/mask
    bilinear weights exactly; one patch index u*131+v per (n,pos) with
    u,v = clamp(floor(p)+1, 0, 130) into a host-built edge-replicated
    132x132 "2x2 patch table" (entry = 4 corner pixels x 64 ch, 512 B).
 3. dma_gather (non-transpose mode, bf16, 512B tokens) runs its descriptor
    generation on 4 SWDGE queues round-robin = 4 concurrent Q7 core pairs
    (the transpose-gather sprays share xbar state and cannot overlap, but
    plain CME writes can). Tokens land position-major: [pos%128, slot=(n,
    chunk)*256 + (r,pp,c)].
 4. One HWDGE xbar DMA-transpose per half-group flips to channel-major
    G2[(pp,c), b*256 + r*128 + p] — matmul-ready.
 5. Corner weights mm*gx*gy are built per half-group in the matching
    interleaved order, partition-broadcast by doubling DMAs, applied with
    two contiguous bf16 tensor_tensor multiplies.
 6. The 4-corner bilinear sum AND the final 3x3 (stride-3) conv collapse
    into 18 PE matmuls with K=(pp,c)=128 per (r,n) tile (f32 PSUM accum).

The gather table, weight repacks, and coordinate base planes are prepared on
the host in numpy (layout/sharding prep only — no FLOPs on tensor data other
than the bf16 cast of the table).
"""

import numpy as np
import ml_dtypes

# ---- problem constants (hardcoded per contract) ----
B, C, H, W = 2, 64, 128, 128
KS, N, DIL, PAD = 3, 9, 2, 1
Hp = H + 2 * PAD            # 130
EXT = Hp + 2                # 132 (edge-replicated ext rows/cols)
NPATCH = 131 * 131          # patch-table entries (2x2 windows of ext image)
NCORES = 8
HSH = H // 4                # 32 rows per core
NPOS = HSH * W              # 4096 positions per core
NG = 4                      # groups per core
GPOS = NPOS // NG           # 1024 positions per group
NBLK = GPOS // 128          # 8 pos-blocks of 128 per group
M_CONV = 73                 # fused conv rows: off 0:18 | ad 32:50 | m 64:73
HPOS = 512                  # positions per half-group
HTOK = HPOS * N             # 4608 patch tokens per half-group
QTOK = HTOK // 2            # 2304 tokens per queue-gather

_cache = {}


# ======================================================================
# host-side input preparation
# ======================================================================

def _prep_consts(w_p, b_p, w_m, w_ad, w_conv):
    f32 = np.float32
    # fused conv taps: wt[t, c, m], t = dy*3+dx
    wt = np.zeros((9, C, M_CONV), f32)
    rep3 = [0, 1, 2, 0, 1, 2, 0, 1, 2]
    for t in range(9):
        dy, dx = t // 3, t % 3
        wt[t, :, 0:9] = w_p[0:9, :, dy, dx].T
        wt[t, :, 9:18] = w_p[9:18, :, dy, dx].T
        wt[t, :, 32:41] = w_ad[rep3, :, dy, dx].T
        wt[t, :, 41:50] = w_ad[rep3, :, dy, dx].T
        wt[t, :, 64:73] = w_m[0:9, :, dy, dx].T
    wt = np.ascontiguousarray(wt.transpose(1, 0, 2).reshape(C, 9 * M_CONV))

    bp2 = b_p.reshape(18, 1).astype(f32)

    r = np.array([-1.0, 0.0, 1.0], f32)
    pnx = np.repeat(r, 3)        # [-1,-1,-1,0,0,0,1,1,1]
    pny = np.tile(r, 3)          # [-1,0,1,-1,0,1,-1,0,1]
    pn18 = np.concatenate([pnx, pny]).reshape(18, 1).astype(f32)

    # idx matmul matrix: [18, 9]: idx = 131*R1x + R1y
    lconst = np.zeros((18, 9), f32)
    for n in range(9):
        lconst[n, n] = 131.0
        lconst[9 + n, n] = 1.0
    # w3: [128=(pp,c), 18*64] bf16, k-tile t=(r*9+n) cols t*64:(t+1)*64
    w3 = np.zeros((128, 18 * 64), f32)
    for t in range(18):
        n = t % 9
        blk = w_conv[:, :, n // 3, n % 3].T  # [c, o]
        w3[0:64, t * 64:(t + 1) * 64] = blk
        w3[64:128, t * 64:(t + 1) * 64] = blk
    w3 = w3.astype(ml_dtypes.bfloat16)

    ident = np.eye(128, dtype=f32)
    return dict(wt=wt, bp2=bp2, pn18=pn18, lconst=lconst, w3=w3, ident=ident)


def _prep_table(xb):
    """xb: [C, H, W] f32 -> patch table [NPATCH, 256] bf16.

    Entry (u, v) (u, v in [0, 130], idx = u*131+v) holds the 2x2 pixel
    patch of the edge-replicated 132x132 padded image at rows (u, u+1),
    cols (v, v+1), laid out (r, pp, c)."""
    xp = np.pad(xb, ((0, 0), (PAD, PAD), (PAD, PAD)))          # [C, 130, 130]
    idx = np.clip(np.arange(EXT) - 1, 0, Hp - 1)
    ext = xp[:, idx][:, :, idx]                                # [C, 132, 132]
    win = np.lib.stride_tricks.sliding_window_view(ext, (2, 2), axis=(1, 2))
    # win: [C, 131, 131, 2, 2] -> (u, v, r, pp, c)
    patch = np.ascontiguousarray(win.transpose(1, 2, 3, 4, 0)).reshape(NPATCH, 256)
    return patch.astype(ml_dtypes.bfloat16)


def _prep_core_inputs(core, x, consts):
    b, hc = core // 4, core % 4
    h0 = hc * HSH
    # conv input rows h0-1 .. h0+32 (34 rows), zero padded at batch edges
    xs = np.zeros((C, HSH + 2, W), np.float32)
    lo, hi = h0 - 1, h0 + HSH + 1
    slo, shi = max(lo, 0), min(hi, H)
    xs[:, slo - lo:shi - lo, :] = x[b, :, slo:shi, :]

    base = np.zeros((18, NPOS), np.float32)
    pos = np.arange(NPOS)
    base[0:9, :] = (h0 + pos // W + 1)[None, :]
    base[9:18, :] = (pos % W + 1)[None, :]

    m = dict(xs=xs, base18=base, xe=_cache[('xe', b)])
    m.update({k: consts[k] for k in ('wt', 'bp2', 'pn18', 'lconst', 'w3', 'ident')})
    return m


# ======================================================================
# bass program
# ======================================================================

def _emit(nc, tc, t):
    import concourse.bass as bass
    import concourse.mybir as mybir
    from concourse.bass import AP

    dt = mybir.dt
    ALU = mybir.AluOpType
    ACTF = mybir.ActivationFunctionType
    f32, bf16, i16, i32 = dt.float32, dt.bfloat16, dt.int16, dt.int32

    XROW = HSH + 2          # 34
    XCW = W + 2             # 130 padded row width in sbuf
    any_, vec, act, pe, gp, snc = nc.any, nc.vector, nc.scalar, nc.tensor, nc.gpsimd, nc.sync

    with tc.tile_pool(name="const", bufs=1) as cpool, \
         tc.tile_pool(name="work", bufs=1) as wpool, \
         tc.tile_pool(name="gath", bufs=2) as gpool, \
         tc.tile_pool(name="gath2", bufs=2) as g2pool, \
         tc.tile_pool(name="psA", bufs=2, space="PSUM") as psA, \
         tc.tile_pool(name="psB", bufs=2, space="PSUM") as psB:

        # ---- load constants / inputs to SBUF ----
        xsb = cpool.tile([C, XROW * XCW], f32, tag="xsb")
        gp.memset(xsb[:], 0.0)
        snc.dma_start(
            out=AP(xsb.tensor, xsb[:].offset + 1,
                   [[XROW * XCW, C], [XCW, XROW], [1, W]]),
            in_=t['xs'][:])
        wtb = cpool.tile([C, 9 * M_CONV], f32, tag="wtb")
        snc.dma_start(out=wtb[:], in_=t['wt'][:])
        w3b = cpool.tile([128, 18 * 64], bf16, tag="w3b")
        snc.dma_start(out=w3b[:], in_=t['w3'][:])
        idb = cpool.tile([18, 18], f32, tag="idb")
        snc.dma_start(out=idb[:], in_=t['ident'][:18, :18])
        bpb = cpool.tile([18, 1], f32, tag="bpb")
        snc.dma_start(out=bpb[:], in_=t['bp2'][:])
        pnb = cpool.tile([18, 1], f32, tag="pnb")
        snc.dma_start(out=pnb[:], in_=t['pn18'][:])
        lcb = cpool.tile([18, 9], f32, tag="lcb")
        snc.dma_start(out=lcb[:], in_=t['lconst'][:])
        mbias = cpool.tile([18, 1], f32, tag="mbias")
        gp.memset(mbias[:], -64.5)

        xe = t['xe']  # dram [NPATCH, 256] bf16

        for g in range(NG):
            # ---------------- fused conv ----------------
            conv = wpool.tile([M_CONV, GPOS], f32, tag="conv")
            for k in range(2):
                pc = psA.tile([M_CONV, 512], f32, tag="pc")
                for tap in range(9):
                    dy, dx = tap // 3, tap % 3
                    rhs = AP(xsb.tensor,
                             xsb[:].offset + (g * 8 + k * 4 + dy) * XCW + dx,
                             [[XROW * XCW, C], [XCW, 4], [1, W]])
                    pe.matmul(pc[:], wtb[:, tap * M_CONV:(tap + 1) * M_CONV],
                              rhs, start=(tap == 0), stop=(tap == 8))
                sl = slice(k * 512, (k + 1) * 512)
                # rows 0:18 offsets + bias; 18:36 -> 1-sigmoid; 36:45 -> sigmoid
                act.activation(conv[0:18, sl], pc[0:18, :], ACTF.Identity,
                               bias=bpb[:], scale=1.0)
                act.activation(conv[32:50, sl], pc[32:50, :], ACTF.Sigmoid,
                               scale=-1.0)
                act.activation(conv[64:73, sl], pc[64:73, :], ACTF.Sigmoid,
                               scale=1.0)

            OFF, SGN, SGM = conv[0:18, :], conv[32:50, :], conv[64:73, :]
            bsl = wpool.tile([18, GPOS], f32, tag="bsl")
            snc.dma_start(out=bsl[:], in_=t['base18'][:, g * GPOS:(g + 1) * GPOS])

            # ---------------- coordinates (f32) ----------------
            # ADF = 1 + 2*(1-sigmoid) written over the SGN rows in place
            vec.tensor_scalar(SGN, SGN, 2.0, 1.0, ALU.mult, ALU.add)
            # base-0 copies (walrus: TensorScalarPtr/TensorTensor SBUF inputs
            # must share base partition; DMA is exempt)
            ADF = wpool.tile([18, GPOS], f32, tag="adf0")
            snc.dma_start(out=ADF[:], in_=SGN)
            SGM0 = wpool.tile([9, GPOS], f32, tag="sgm0")
            snc.dma_start(out=SGM0[:], in_=SGM)
            V = wpool.tile([18, GPOS], f32, tag="v")
            vec.tensor_tensor(V[:], OFF, bsl[:], op=ALU.add)
            vec.scalar_tensor_tensor(V[:], ADF[:], pnb[:], V[:],
                                     op0=ALU.mult, op1=ALU.add)
            # floor(V) robust to convert rounding mode: g=int(V); F=g-(g>V)
            I32T = wpool.tile([18, GPOS], i32, tag="i32t")
            vec.tensor_copy(I32T[:], V[:])
            F = wpool.tile([18, GPOS], f32, tag="f")
            vec.tensor_copy(F[:], I32T[:])
            FRAC = wpool.tile([18, GPOS], f32, tag="frac")
            vec.tensor_tensor(FRAC[:], F[:], V[:], op=ALU.is_gt)
            vec.tensor_tensor(F[:], F[:], FRAC[:], op=ALU.subtract)
            vec.tensor_tensor(FRAC[:], V[:], F[:], op=ALU.subtract)
            QRB1 = wpool.tile([18, GPOS], f32, tag="qrb1")
            vec.tensor_scalar(QRB1[:], F[:], 1.0, 0.0, ALU.add, ALU.max)
            # QLT over F in place
            vec.tensor_scalar(F[:], F[:], 0.0, Hp - 1.0, ALU.max, ALU.min)
            QLT = F
            R1 = wpool.tile([18, GPOS], f32, tag="r1")
            vec.tensor_scalar(R1[:], QRB1[:], Hp + 0.0, None, ALU.min)
            # QRB over QRB1 in place (after R1 consumed the pre-min value)
            vec.tensor_scalar(QRB1[:], QRB1[:], Hp - 1.0, None, ALU.min)
            QRB = QRB1
            MASK = wpool.tile([18, GPOS], f32, tag="mask")
            act.activation(MASK[:], V[:], ACTF.Abs, bias=mbias[:], scale=1.0)
            vec.tensor_scalar(MASK[:], MASK[:], 63.5, None, ALU.is_gt)
            vec.tensor_tensor(MASK[:], MASK[:], FRAC[:], op=ALU.mult)
            # PF = clamp(V - mask*frac) stored into V
            vec.tensor_tensor(V[:], V[:], MASK[:], op=ALU.subtract)
            vec.tensor_scalar(V[:], V[:], 0.0, Hp - 1.0, ALU.max, ALU.min)
            GLT = wpool.tile([18, GPOS], f32, tag="glt")
            vec.scalar_tensor_tensor(GLT[:], QLT[:], 1.0, V[:],
                                     op0=ALU.add, op1=ALU.subtract)
            GRB = wpool.tile([18, GPOS], f32, tag="grb")
            vec.scalar_tensor_tensor(GRB[:], V[:], 1.0, QRB[:],
                                     op0=ALU.add, op1=ALU.subtract)

            # ---------------- modulation & corner weights ----
            ADM = wpool.tile([9, GPOS], f32, tag="adm")
            vec.tensor_scalar(ADM[:], ADF[0:9, :], 2.0, -4.0, ALU.mult, ALU.add)
            vec.tensor_tensor(ADM[:], SGM0[:], ADM[:], op=ALU.mult)  # ADM = mm
            T1 = wpool.tile([9, GPOS], f32, tag="t1")
            vec.tensor_tensor(T1[:], ADM[:], GLT[0:9, :], op=ALU.mult)
            T2 = wpool.tile([9, GPOS], f32, tag="t2")
            vec.tensor_tensor(T2[:], ADM[:], GRB[0:9, :], op=ALU.mult)
            # y-halves moved to partition-0-based tiles (engines need aligned
            # partition starts; DMA is exempt from that rule)
            GLTY = wpool.tile([9, GPOS], f32, tag="glty")
            snc.dma_start(out=GLTY[:], in_=GLT[9:18, :])
            GRBY = wpool.tile([9, GPOS], f32, tag="grby")
            snc.dma_start(out=GRBY[:], in_=GRB[9:18, :])

            # ---------------- patch indices ----------------
            # idxf col (bl*128 + p*8 + s) <- idx[n, pos=bl*128+s*16+p]
            idxf = wpool.tile([9, GPOS], f32, tag="idxf")
            for k in range(2):
                pi = psB.tile([9, 512], f32, tag="pi")
                pe.matmul(pi[:], lcb[:], R1[:, k * 512:(k + 1) * 512],
                          start=True, stop=True)
                act.activation(
                    AP(idxf.tensor, idxf[:].offset + k * 512,
                       [[GPOS, 9], [128, 4], [1, 8], [8, 16]]),
                    pi[:], ACTF.Copy, scale=1.0)
            # idxt[p*8+s, bl*9+n] = idx[n, pos=bl*128+s*16+p] (i16)
            idxt = wpool.tile([128, NBLK * 9], i16, tag="idxt")
            for bl in range(NBLK):
                tp = psB.tile([128, 9], f32, tag="tp")
                pe.transpose(tp[:], idxf[:, bl * 128:(bl + 1) * 128],
                             idb[0:9, 0:9])
                vec.tensor_copy(idxt[:, bl * 9:bl * 9 + 9], tp[:])

            for h in range(2):
                # corner weights for this half-group, written in the
                # G2-matching interleaved order col = n*1024 + chunk*256 + pos%128
                # handled by the WRB seed DMA below; WC stays [9, 512].
                sl5 = slice(h * 512, (h + 1) * 512)
                WC = []
                for (srcw, gy, tag) in ((T1, GLTY, "w00"), (T1, GRBY, "w01"),
                                        (T2, GLTY, "w10"), (T2, GRBY, "w11")):
                    wc = wpool.tile([9, 512], bf16, tag=tag)
                    vec.tensor_tensor(wc[:], srcw[0:9, sl5], gy[0:9, sl5],
                                      op=ALU.mult)
                    WC.append(wc)

                # idx shuffle, two hops (DMA balancer wants descending-stride
                # free dims; the final n-major permute runs on DVE instead):
                #   idxm[p, (bl*8+s)*9 + n] = idxt[p*8+s, (h*4+bl)*9+n]
                #   idxw[p, n*32 + w]       = idxm[p, w*9 + n]
                idxm = wpool.tile([16, HTOK // 16], i16, tag="idxm")
                for bl in range(4):
                    snc.dma_start(
                        out=AP(idxm.tensor, idxm[:].offset + bl * 72,
                               [[HTOK // 16, 16], [9, 8], [1, 9]]),
                        in_=AP(idxt.tensor,
                               idxt[:].offset + (h * 4 + bl) * 9,
                               [[NBLK * 9, 128], [1, 9]]))
                idxw = wpool.tile([16, HTOK // 16], i16, tag="idxw")
                vec.tensor_copy(
                    AP(idxw.tensor, idxw[:].offset,
                       [[HTOK // 16, 16], [32, 9], [1, 32]]),
                    AP(idxm.tensor, idxm[:].offset,
                       [[HTOK // 16, 16], [1, 9], [9, 32]]))
                idxr = wpool.tile([128, HTOK // 16], i16, tag="idxr")
                snc.dma_start(out=idxr[0:16, :], in_=idxw[:])
                snc.dma_start(out=idxr[16:32, :], in_=idxr[0:16, :])
                snc.dma_start(out=idxr[32:64, :], in_=idxr[0:32, :])
                snc.dma_start(out=idxr[64:128, :], in_=idxr[0:64, :])

                # ---------------- gather (non-transpose, 4 queues) --------
                # token t = n*512 + pos_h -> partition t%128, slot t//128;
                # slot content = 256 elems (r, pp, c).
                Gp = gpool.tile([128, 36 * 256], bf16, tag="Gp")
                for qh in range(2):
                    qn = (2 * (g * 2 + h) + qh) % 4
                    gp.dma_gather(
                        out_ap=AP(Gp.tensor, Gp[:].offset + qh * 18 * 256,
                                  [[36 * 256, 128], [256, 18], [1, 256]]),
                        in_ap=xe[:],
                        idxs_ap=idxr[:, qh * (QTOK // 16):(qh + 1) * (QTOK // 16)],
                        num_idxs=QTOK,
                        num_idxs_reg=QTOK,
                        elem_size=256,
                        elem_step=256,
                        transpose=False,
                        single_packet=False,
                        queue_num=qn)

                # ---------------- xbar transpose to channel-major ---------
                # G2[(pp,c), b*256 + r*128 + p] = Gp[p, b*256 + r*128 + (pp,c)]
                G2 = g2pool.tile([128, 2 * HTOK], bf16, tag="G2")
                snc.dma_start(
                    out=AP(G2.tensor, G2[:].offset,
                           [[2 * HTOK, 128], [128, 72], [1, 128]]),
                    in_=Gp[:],
                    transpose=True)

                # ---------------- corner-weight broadcast ----------------
                # WRB[(pp,c), (n*4+chunk)*256 + r*128 + p] = wc_{r,pp}[n, chunk*128+p]
                WRB = wpool.tile([128, 2 * HTOK], bf16, tag="wrb")
                PIT = 2 * HTOK
                for r in range(2):
                    for pp in range(2):
                        snc.dma_start(
                            out=AP(WRB.tensor,
                                   WRB[:].offset + pp * 64 * PIT + r * 128,
                                   [[PIT, 1], [1024, 9], [256, 4], [1, 128]]),
                            in_=WC[r * 2 + pp][:])
                # doubling across partitions
                for kk in (1, 2, 4, 8, 16, 32):
                    for pp in range(2):
                        sl_src = AP(WRB.tensor,
                                    WRB[:].offset + pp * 64 * PIT,
                                    [[PIT, kk], [1, PIT]])
                        sl_dst = AP(WRB.tensor,
                                    WRB[:].offset + (pp * 64 + kk) * PIT,
                                    [[PIT, kk], [1, PIT]])
                        snc.dma_start(out=sl_dst, in_=sl_src)

                # ---------------- apply weights (in place) ----------------
                for hh in range(2):
                    slh = slice(hh * HTOK, (hh + 1) * HTOK)
                    any_.tensor_tensor(G2[:, slh], G2[:, slh], WRB[:, slh],
                                       op=ALU.mult)

                # ---------------- final matmuls ----------------
                po = psA.tile([64, 512], f32, tag="po")
                for tt in range(18):
                    r_, n_ = tt // 9, tt % 9
                    rhs = AP(G2.tensor,
                             G2[:].offset + n_ * 1024 + r_ * 128,
                             [[2 * HTOK, 128], [256, 4], [1, 128]])
                    pe.matmul(po[:], w3b[:, tt * 64:(tt + 1) * 64], rhs,
                              start=(tt == 0), stop=(tt == 17))
                oc = wpool.tile([64, 512], f32, tag="oc")
                act.activation(oc[:], po[:], ACTF.Copy, scale=1.0)
                snc.dma_start(
                    out=t['outp'][:, g * GPOS + h * 512:g * GPOS + (h + 1) * 512],
                    in_=oc[:])


def _build():
    import concourse.bacc as bacc
    import concourse.tile as tile
    import concourse.mybir as mybir
    dt = mybir.dt

    nc = bacc.Bacc("TRN2", target_bir_lowering=False, debug=False,
                   num_swdge_queues=4)
    t = {}
    specs = [
        ('xs', [C, HSH + 2, W], dt.float32),
        ('xe', [NPATCH, 256], dt.bfloat16),
        ('wt', [C, 9 * M_CONV], dt.float32),
        ('bp2', [18, 1], dt.float32),
        ('pn18', [18, 1], dt.float32),
        ('lconst', [18, 9], dt.float32),
        ('w3', [128, 18 * 64], dt.bfloat16),
        ('ident', [128, 128], dt.float32),
        ('base18', [18, NPOS], dt.float32),
    ]
    for name, shape, d in specs:
        t[name] = nc.dram_tensor(name, shape, d, kind="ExternalInput").ap()
    t['outp'] = nc.dram_tensor('outp', [64, NPOS], dt.float32,
                               kind="ExternalOutput").ap()
    with tile.TileContext(nc) as tc:
        _emit(nc, tc, t)
    nc.compile()
    return nc


def kernel(x, w_p, b_p, w_m, w_ad, w_conv):
    from concourse.bass_utils import run_bass_kernel_spmd

    x = np.asarray(x, np.float32)
    consts = _prep_consts(np.asarray(w_p, np.float32), np.asarray(b_p, np.float32),
                          np.asarray(w_m, np.float32), np.asarray(w_ad, np.float32),
                          np.asarray(w_conv, np.float32))
    for b in range(B):
        _cache[('xe', b)] = _prep_table(x[b])
    if 'nc' not in _cache:
        _cache['nc'] = _build()
    nc = _cache['nc']

    in_maps = [_prep_core_inputs(c, x, consts) for c in range(NCORES)]
    res = run_bass_kernel_spmd(nc, in_maps, list(range(NCORES)))
    _cache['last_results'] = res

    out = np.zeros((B, 64, H, W), np.float32)
    for c in range(NCORES):
        b, hc = c // 4, c % 4
        out[b, :, hc * HSH:(hc + 1) * HSH, :] = \
            res.results[c]['outp'].reshape(64, HSH, W)
    return out
